# revision 11
# baseline (speedup 1.0000x reference)
"""MultiHeadManhattanAttention Trainium2 kernel (8-core SPMD).

Sharding: core c handles batch b = c//2 and query rows i in
[1024*(c%2), 1024*(c%2)+1024), all 8 heads. Everything (projections,
L1-distance scores, softmax, attn@V, output proj, layernorm) is
core-local; host only concatenates per-core outputs.

Layouts on chip:
  Q        (i128-part, hd-free)   per i-tile, bias columns for ACT
  K^T      DRAM scratch (512, 2048); per (h,d) row broadcast to 128
           partitions (krep) via stride-0 DMA
  dist/attn acc tiles (i128-part, j-free 2048), one per i-tile
  V        fp16 (j128-part, hd-free)
  attn^T   fp16 (j128-part, i-free) via PE transpose, feeds attn@V
  attnV^T  (hd-part, i-free) -> output proj -> PE transpose -> (i, hd)
"""
import sys
sys.path.insert(0, "/opt/trn_rl_repo")
import numpy as np
from contextlib import ExitStack

import concourse.bass as bass
from concourse import bacc
import concourse.tile as tile
import concourse.mybir as mybir
from concourse.bass_utils import run_bass_kernel_spmd
from concourse.masks import make_identity

HIDDEN, HEADS, HD = 512, 8, 64
B, S = 4, 2048
SI = 1024            # query rows per core
NIT = SI // 128      # 8 i-tiles
NJT = S // 128       # 16 j-tiles
EPS = 1e-5
SHIFT = 72.0
DVE_DS = {5, 11, 18, 25, 33, 39, 46, 53}  # dims whose |q-k| runs on DVE (not 0 mod 4)         # softmax stability shift: exp(-temp*(dist-SHIFT))

F32 = mybir.dt.float32
F16 = mybir.dt.float16
I32 = mybir.dt.int32
AF = mybir.ActivationFunctionType
OP = mybir.AluOpType


def build_kernel():
    nc = bacc.Bacc("TRN2", target_bir_lowering=False, debug=False, num_devices=8)

    query = nc.dram_tensor("query", [SI, HIDDEN], F32, kind="ExternalInput")
    key = nc.dram_tensor("key", [S, HIDDEN], F32, kind="ExternalInput")
    value = nc.dram_tensor("value", [S, HIDDEN], F32, kind="ExternalInput")
    maskin = nc.dram_tensor("mask", [1, S], I32, kind="ExternalInput")
    Ws = {n: nc.dram_tensor(n, [HIDDEN, HIDDEN], F32, kind="ExternalInput")
          for n in ("Wq", "Wk", "Wv", "Wo")}
    vecs = {n: nc.dram_tensor(n, [1, HIDDEN], F32, kind="ExternalInput")
            for n in ("bq", "bk", "bv", "bo", "gamma", "beta")}
    temp = nc.dram_tensor("temperature", [1, HEADS], F32, kind="ExternalInput")

    out = nc.dram_tensor("out", [SI, HIDDEN], F32, kind="ExternalOutput")
    attn = nc.dram_tensor("attn", [HEADS, SI, S], F32, kind="ExternalOutput")
    kT_dram = nc.dram_tensor("kT_scratch", [HIDDEN, S], F16)

    with tile.TileContext(nc) as tc, ExitStack() as ctx:
        const = ctx.enter_context(tc.tile_pool(name="const", bufs=1))
        qpool = ctx.enter_context(tc.tile_pool(name="qrows", bufs=NIT))
        vpool = ctx.enter_context(tc.tile_pool(name="v16", bufs=NJT))
        avtp = ctx.enter_context(tc.tile_pool(name="avt", bufs=1))

        ident = const.tile([128, 128], F32)
        make_identity(nc, ident[:])

        # ---- replicated small constants ----
        def rep(name, src, n):
            t = const.tile([128, n], F32, tag=name)
            nc.sync.dma_start(t[:], src[0:1, :].broadcast_to([128, n]))
            return t

        bqRep = rep("bqRep", vecs["bq"], HIDDEN)
        bvRep = rep("bvRep", vecs["bv"], HIDDEN)
        gammaRep = rep("gammaRep", vecs["gamma"], HIDDEN)
        betaRep = rep("betaRep", vecs["beta"], HIDDEN)
        tempRep = rep("tempRep", temp, HEADS)
        negTemp = const.tile([128, HEADS], F32)
        nc.vector.tensor_scalar(out=negTemp[:], in0=tempRep[:], scalar1=-1.0,
                                scalar2=None, op0=OP.mult)
        bias72 = const.tile([128, HEADS], F32)
        nc.vector.tensor_scalar(out=bias72[:], in0=tempRep[:], scalar1=SHIFT,
                                scalar2=None, op0=OP.mult)

        # mask -> additive big-penalty row, replicated: (1-m)*1e6
        maskI = const.tile([128, S], I32, tag="maskstage")
        nc.sync.dma_start(maskI[:], maskin[0:1, :].broadcast_to([128, S]))
        maskBig = const.tile([128, S], F32)
        nc.vector.tensor_copy(maskBig[:], maskI[:])
        nc.vector.tensor_scalar(out=maskBig[:], in0=maskBig[:], scalar1=-30000.0,
                                scalar2=30000.0, op0=OP.mult, op1=OP.add)

        # per-partition bias columns for bk / bo: colT[p, c] = vec[c*128+p]
        def colT(name, src):
            t = const.tile([128, 4], F32, tag=name)
            nc.sync.dma_start(
                t[:], src.rearrange("a (c p) -> p (c a)", p=128))
            return t

        bkColT = colT("bkColT", vecs["bk"])
        boColT = colT("boColT", vecs["bo"])

        # ================= Phase A: projections =================
        WT = {}
        with tc.tile_pool(name="phA", bufs=2) as pa, \
             tc.tile_pool(name="phA_wnat", bufs=1) as pwn, \
             tc.tile_pool(name="phA_kT", bufs=1) as pkT, \
             tc.tile_pool(name="phA_ps", bufs=2, space="PSUM") as pps, \
             tc.tile_pool(name="phA_pst", bufs=2, space="PSUM") as ppt:

            # A0: load weights, build W^T (in-part, out-free), 4 chunks each
            for wn in ("Wq", "Wk", "Wv", "Wo"):
                wt_pool = const if wn == "Wo" else pwn
                wts = []
                for ic in range(4):
                    wt = wt_pool.tile([128, HIDDEN], F32, tag=f"{wn}T{ic}", name=f"{wn}T{ic}")
                    wts.append(wt)
                WT[wn] = wts
                for oc in range(4):
                    wnat = pa.tile([128, HIDDEN], F32, tag="wnat")
                    nc.sync.dma_start(wnat[:], Ws[wn][oc * 128:(oc + 1) * 128, :])
                    for ic in range(4):
                        pst = ppt.tile([128, 128], F32, tag="tpsA")
                        nc.tensor.transpose(
                            pst[:], wnat[:, ic * 128:(ic + 1) * 128], ident[:])
                        nc.scalar.copy(
                            WT[wn][ic][:, oc * 128:(oc + 1) * 128], pst[:])

            # A1: Q = query @ Wq.T + bq   -> Q tiles (i128, 512)
            Qt = []
            for it in range(NIT):
                qnat = pa.tile([128, HIDDEN], F32, tag="qnat")
                nc.sync.dma_start(qnat[:], query[it * 128:(it + 1) * 128, :])
                qTt = pa.tile([128, 128 * 4], F32, tag="qT")  # (in, i128) x 4chunks
                for ic in range(4):
                    pst = ppt.tile([128, 128], F32, tag="tpsA")
                    nc.tensor.transpose(
                        pst[:], qnat[:, ic * 128:(ic + 1) * 128], ident[:])
                    nc.scalar.copy(qTt[:, ic * 128:(ic + 1) * 128], pst[:])
                ps = pps.tile([128, HIDDEN], F32, tag="projps")
                for ic in range(4):
                    nc.tensor.matmul(ps[:], qTt[:, ic * 128:(ic + 1) * 128],
                                     WT["Wq"][ic][:],
                                     start=(ic == 0), stop=(ic == 3))
                q_sb = qpool.tile([128, HIDDEN], F32, tag="Q")
                nc.vector.tensor_tensor(out=q_sb[:], in0=ps[:], in1=bqRep[:],
                                        op=OP.add)
                Qt.append(q_sb)

            # A2: keyT chunks (in-part, j-free), then K^T = Wk@key^T + bk
            #     (+ mask fold) streamed to DRAM scratch
            keyT = [pkT.tile([128, S], F32, tag=f"keyT{ic}", name=f"keyT{ic}")
                    for ic in range(4)]
            for jt in range(NJT):
                knat = pa.tile([128, HIDDEN], F32, tag="knat")
                nc.sync.dma_start(knat[:], key[jt * 128:(jt + 1) * 128, :])
                for ic in range(4):
                    pst = ppt.tile([128, 128], F32, tag="tpsA")
                    nc.tensor.transpose(
                        pst[:], knat[:, ic * 128:(ic + 1) * 128], ident[:])
                    nc.scalar.copy(
                        keyT[ic][:, jt * 128:(jt + 1) * 128], pst[:])
            for oc in range(4):
                for jc in range(4):
                    ps = pps.tile([128, 512], F32, tag="projps")
                    for ic in range(4):
                        nc.tensor.matmul(
                            ps[:],
                            WT["Wk"][ic][:, oc * 128:(oc + 1) * 128],
                            keyT[ic][:, jc * 512:(jc + 1) * 512],
                            start=(ic == 0), stop=(ic == 3))
                    kT_sb = pa.tile([128, 512], F16, tag="kTsb")
                    nc.scalar.activation(kT_sb[:], ps[:], AF.Identity,
                                         bias=bkColT[:, oc:oc + 1])
                    nc.vector.tensor_tensor(
                        out=kT_sb[:], in0=kT_sb[:],
                        in1=maskBig[:, jc * 512:(jc + 1) * 512], op=OP.add)
                    nc.sync.dma_start(
                        kT_dram[oc * 128:(oc + 1) * 128,
                                jc * 512:(jc + 1) * 512], kT_sb[:])

            # A3: V (j-part, hd-free) fp16; reuse keyT slots for valueT
            valueT = [pkT.tile([128, S], F32, tag=f"keyT{ic}", name=f"valT{ic}")
                      for ic in range(4)]
            for jt in range(NJT):
                vnat = pa.tile([128, HIDDEN], F32, tag="knat")
                nc.sync.dma_start(vnat[:], value[jt * 128:(jt + 1) * 128, :])
                for ic in range(4):
                    pst = ppt.tile([128, 128], F32, tag="tpsA")
                    nc.tensor.transpose(
                        pst[:], vnat[:, ic * 128:(ic + 1) * 128], ident[:])
                    nc.scalar.copy(
                        valueT[ic][:, jt * 128:(jt + 1) * 128], pst[:])
            V16 = []
            for jt in range(NJT):
                ps = pps.tile([128, HIDDEN], F32, tag="projps")
                for ic in range(4):
                    nc.tensor.matmul(
                        ps[:],
                        valueT[ic][:, jt * 128:(jt + 1) * 128],
                        WT["Wv"][ic][:],
                        start=(ic == 0), stop=(ic == 3))
                v_sb = vpool.tile([128, HIDDEN], F16, tag="V16")
                nc.vector.tensor_tensor(out=v_sb[:], in0=ps[:], in1=bvRep[:],
                                        op=OP.add)
                V16.append(v_sb)

        # ============ Phases B+C per head: dist, softmax, attn, PV ============
        attnVT = [avtp.tile([128, SI], F32, tag=f"avt{c}", name=f"avt{c}")
              for c in range(4)]
        with tc.tile_pool(name="acc", bufs=1) as paccp, \
             tc.tile_pool(name="krep", bufs=2) as pkrep, \
             tc.tile_pool(name="tmp", bufs=2) as ptmp, \
             tc.tile_pool(name="sm", bufs=8) as psm, \
             tc.tile_pool(name="at16", bufs=4) as pat, \
             tc.tile_pool(name="psT", bufs=3, space="PSUM") as ppsT, \
             tc.tile_pool(name="psPV", bufs=2, space="PSUM") as ppsPV:
            for h in range(HEADS):
                acc = [paccp.tile([128, S], F32, tag=f"acc{it}", name=f"acc{it}")
                       for it in range(NIT)]
                acclo = [paccp.tile([128, S], F16, tag=f"acclo{it}",
                                    name=f"acclo{it}") for it in range(NIT)]
                # --- B: dist[i,j] = sum_d |Q[i,hd] - K[j,hd]| ---
                # fp16 partial sums over groups of 4 dims (DVE 2x mode),
                # flushed into the fp32 acc once per group.
                for d in range(HD):
                    hd = h * HD + d
                    g = d % 4
                    krep = pkrep.tile([128, S], F16, tag="krep")
                    nc.sync.dma_start(
                        krep[:], kT_dram[hd:hd + 1, :].broadcast_to([128, S]))
                    on_dve = d in DVE_DS
                    for it in range(NIT):
                        qcol = Qt[it][:, hd:hd + 1]
                        if g == 0:
                            nc.scalar.activation(acclo[it][:], krep[:], AF.Abs,
                                                 bias=qcol, scale=-1.0)
                        else:
                            tmp = ptmp.tile([128, S], F16, tag="tmp")
                            if on_dve:
                                # |k - q| = max(diff, -diff), all on DVE
                                diff = ptmp.tile([128, S], F16, tag="diff")
                                nc.vector.tensor_scalar(
                                    out=diff[:], in0=krep[:], scalar1=qcol,
                                    scalar2=None, op0=OP.subtract)
                                nc.vector.tensor_scalar(
                                    out=tmp[:], in0=diff[:], scalar1=-1.0,
                                    scalar2=None, op0=OP.mult)
                                nc.vector.tensor_tensor(
                                    out=tmp[:], in0=diff[:], in1=tmp[:],
                                    op=OP.max)
                            else:
                                nc.scalar.activation(tmp[:], krep[:], AF.Abs,
                                                     bias=qcol, scale=-1.0)
                            nc.vector.tensor_tensor(
                                out=acclo[it][:], in0=acclo[it][:], in1=tmp[:],
                                op=OP.add)
                        if g == 3:
                            if d == 3:
                                nc.vector.tensor_copy(acc[it][:], acclo[it][:])
                            else:
                                nc.vector.tensor_tensor(
                                    out=acc[it][:], in0=acc[it][:],
                                    in1=acclo[it][:], op=OP.add)
                # --- C: softmax -> attn out; attn^T @ ... -> attnV^T ---
                for it in range(NIT):
                    denom = psm.tile([128, 1], F32, tag="denom")
                    nc.scalar.activation(acc[it][:], acc[it][:], AF.Exp,
                                         bias=bias72[:, h:h + 1],
                                         scale=negTemp[:, h:h + 1],
                                         accum_out=denom[:])
                    rden = psm.tile([128, 1], F32, tag="rden")
                    nc.vector.reciprocal(rden[:], denom[:])
                    nc.vector.tensor_scalar(out=acc[it][:], in0=acc[it][:],
                                            scalar1=rden[:, 0:1], scalar2=None,
                                            op0=OP.mult)
                    nc.sync.dma_start(
                        attn[h, it * 128:(it + 1) * 128, :], acc[it][:])
                    pv = ppsPV.tile([64, 128], F32, tag="pv")
                    for jt in range(NJT):
                        psT = ppsT.tile([128, 128], F32, tag="psT")
                        nc.tensor.transpose(
                            psT[:], acc[it][:, jt * 128:(jt + 1) * 128],
                            ident[:])
                        aT = pat.tile([128, 128], F16, tag="aT")
                        nc.scalar.copy(aT[:], psT[:])
                        nc.tensor.matmul(
                            pv[:], V16[jt][:, h * HD:(h + 1) * HD], aT[:],
                            start=(jt == 0), stop=(jt == NJT - 1))
                    c, r = (h * HD) // 128, (h * HD) % 128
                    nc.vector.tensor_copy(
                        attnVT[c][r:r + HD, it * 128:(it + 1) * 128], pv[:])

        # ============ Phase D: output proj + residual + layernorm ============
        with tc.tile_pool(name="phD", bufs=2) as pd, \
             tc.tile_pool(name="phD_oT", bufs=1) as poT, \
             tc.tile_pool(name="phD_sc", bufs=4) as psc, \
             tc.tile_pool(name="phD_ps", bufs=2, space="PSUM") as pdps, \
             tc.tile_pool(name="phD_pst", bufs=3, space="PSUM") as pdpt:
            # O^T tiles (o128, i512): O^T = Wo @ attnV^T (+bo)
            oT = []
            for oc in range(4):
                row = []
                for icn in range(2):
                    ps = pdps.tile([128, 512], F32, tag="ops")
                    for c in range(4):
                        nc.tensor.matmul(
                            ps[:],
                            WT["Wo"][c][:, oc * 128:(oc + 1) * 128],
                            attnVT[c][:, icn * 512:(icn + 1) * 512],
                            start=(c == 0), stop=(c == 3))
                    t = poT.tile([128, 512], F32, tag=f"oT{oc}_{icn}")
                    nc.scalar.activation(t[:], ps[:], AF.Identity,
                                         bias=boColT[:, oc:oc + 1])
                    row.append(t)
                oT.append(row)
            # transpose back to (i, hd), residual, LN
            for it in range(NIT):
                icn, sub = it // 4, it % 4
                res = pd.tile([128, HIDDEN], F32, tag="res")
                for oc in range(4):
                    pst = pdpt.tile([128, 128], F32, tag="tpsD")
                    nc.tensor.transpose(
                        pst[:], oT[oc][icn][:, sub * 128:(sub + 1) * 128],
                        ident[:])
                    nc.scalar.copy(res[:, oc * 128:(oc + 1) * 128], pst[:])
                qres = pd.tile([128, HIDDEN], F32, tag="qres")
                nc.sync.dma_start(qres[:], query[it * 128:(it + 1) * 128, :])
                nc.vector.tensor_tensor(out=res[:], in0=res[:], in1=qres[:],
                                        op=OP.add)
                # layernorm over hd
                scr = pd.tile([128, HIDDEN], F32, tag="scr")
                sx = psc.tile([128, 1], F32, tag="sx")
                nc.scalar.activation(scr[:], res[:], AF.Identity,
                                     accum_out=sx[:])
                sx2 = psc.tile([128, 1], F32, tag="sx2")
                nc.scalar.activation(scr[:], res[:], AF.Square,
                                     accum_out=sx2[:])
                mu = psc.tile([128, 1], F32, tag="mu")
                nc.vector.tensor_scalar(out=mu[:], in0=sx[:],
                                        scalar1=1.0 / HIDDEN, scalar2=None,
                                        op0=OP.mult)
                ex2 = psc.tile([128, 1], F32, tag="ex2")
                nc.vector.tensor_scalar(out=ex2[:], in0=sx2[:],
                                        scalar1=1.0 / HIDDEN, scalar2=None,
                                        op0=OP.mult)
                musq = psc.tile([128, 1], F32, tag="musq")
                nc.vector.tensor_scalar(out=musq[:], in0=mu[:],
                                        scalar1=mu[:, 0:1], scalar2=None,
                                        op0=OP.mult)
                var = psc.tile([128, 1], F32, tag="var")
                nc.vector.tensor_tensor(out=var[:], in0=ex2[:], in1=musq[:],
                                        op=OP.subtract)
                veps = psc.tile([128, 1], F32, tag="veps")
                nc.vector.tensor_scalar(out=veps[:], in0=var[:], scalar1=EPS,
                                        scalar2=None, op0=OP.add)
                std = psc.tile([128, 1], F32, tag="std")
                nc.scalar.activation(std[:], veps[:], AF.Sqrt)
                rstd = psc.tile([128, 1], F32, tag="rstd")
                nc.vector.reciprocal(rstd[:], std[:])
                nc.vector.tensor_scalar(out=res[:], in0=res[:],
                                        scalar1=mu[:, 0:1], scalar2=None,
                                        op0=OP.subtract)
                nc.vector.tensor_scalar(out=res[:], in0=res[:],
                                        scalar1=rstd[:, 0:1], scalar2=None,
                                        op0=OP.mult)
                nc.vector.tensor_tensor(out=res[:], in0=res[:], in1=gammaRep[:],
                                        op=OP.mult)
                nc.vector.tensor_tensor(out=res[:], in0=res[:], in1=betaRep[:],
                                        op=OP.add)
                nc.sync.dma_start(out[it * 128:(it + 1) * 128, :], res[:])

    nc.compile()
    return nc


_NC = None


def _get_nc():
    global _NC
    if _NC is None:
        _NC = build_kernel()
    return _NC


def make_in_maps(inputs):
    in_maps = []
    for c in range(8):
        b, half = c // 2, c % 2
        i0 = half * SI
        in_maps.append({
            "query": np.ascontiguousarray(inputs["query"][b, i0:i0 + SI]),
            "key": np.ascontiguousarray(inputs["key"][b]),
            "value": np.ascontiguousarray(inputs["value"][b]),
            "mask": np.ascontiguousarray(inputs["mask"][b][None, :]),
            "Wq": inputs["Wq"], "Wk": inputs["Wk"],
            "Wv": inputs["Wv"], "Wo": inputs["Wo"],
            "bq": inputs["bq"][None, :], "bk": inputs["bk"][None, :],
            "bv": inputs["bv"][None, :], "bo": inputs["bo"][None, :],
            "gamma": inputs["gamma"][None, :], "beta": inputs["beta"][None, :],
            "temperature": inputs["temperature"][None, :],
        })
    return in_maps


def kernel(**inputs):
    nc = _get_nc()
    res = run_bass_kernel_spmd(nc, make_in_maps(inputs), list(range(8)))
    out = np.empty((B, S, HIDDEN), np.float32)
    attn = np.empty((B, HEADS, S, S), np.float32)
    for c in range(8):
        b, half = c // 2, c % 2
        i0 = half * SI
        out[b, i0:i0 + SI] = res.results[c]["out"]
        attn[b, :, i0:i0 + SI, :] = res.results[c]["attn"]
    return out, attn


# revision 14
# speedup vs baseline: 1.0963x; 1.0963x over previous
"""MultiHeadManhattanAttention Trainium2 kernel (8-core SPMD).

Sharding: core c handles batch b = c//2 and query rows i in
[1024*(c%2), 1024*(c%2)+1024), all 8 heads. Everything (projections,
L1-distance scores, softmax, attn@V, output proj, layernorm) is
core-local; host only concatenates per-core outputs.

Layouts on chip:
  Q        (i128-part, hd-free)   per i-tile, bias columns for ACT
  K^T      DRAM scratch (512, 2048); per (h,d) row broadcast to 128
           partitions (krep) via stride-0 DMA
  dist/attn acc tiles (i128-part, j-free 2048), one per i-tile
  V        fp16 (j128-part, hd-free)
  attn^T   fp16 (j128-part, i-free) via PE transpose, feeds attn@V
  attnV^T  (hd-part, i-free) -> output proj -> PE transpose -> (i, hd)
"""
import sys
sys.path.insert(0, "/opt/trn_rl_repo")
import numpy as np
from contextlib import ExitStack

import concourse.bass as bass
from concourse import bacc
import concourse.tile as tile
import concourse.mybir as mybir
from concourse.bass_utils import run_bass_kernel_spmd
from concourse.masks import make_identity

HIDDEN, HEADS, HD = 512, 8, 64
B, S = 4, 2048
SI = 1024            # query rows per core
NIT = SI // 128      # 8 i-tiles
NJT = S // 128       # 16 j-tiles
EPS = 1e-5
SHIFT = 72.0         # softmax stability shift: exp(-temp*(dist-SHIFT))

F32 = mybir.dt.float32
F16 = mybir.dt.float16
I32 = mybir.dt.int32
AF = mybir.ActivationFunctionType
OP = mybir.AluOpType


def build_kernel():
    nc = bacc.Bacc("TRN2", target_bir_lowering=False, debug=False, num_devices=8)

    query = nc.dram_tensor("query", [SI, HIDDEN], F32, kind="ExternalInput")
    key = nc.dram_tensor("key", [S, HIDDEN], F32, kind="ExternalInput")
    value = nc.dram_tensor("value", [S, HIDDEN], F32, kind="ExternalInput")
    maskin = nc.dram_tensor("mask", [1, S], I32, kind="ExternalInput")
    Ws = {n: nc.dram_tensor(n, [HIDDEN, HIDDEN], F32, kind="ExternalInput")
          for n in ("Wq", "Wk", "Wv", "Wo")}
    vecs = {n: nc.dram_tensor(n, [1, HIDDEN], F32, kind="ExternalInput")
            for n in ("bq", "bk", "bv", "bo", "gamma", "beta")}
    temp = nc.dram_tensor("temperature", [1, HEADS], F32, kind="ExternalInput")

    out = nc.dram_tensor("out", [SI, HIDDEN], F32, kind="ExternalOutput")
    attn = nc.dram_tensor("attn", [HEADS, SI, S], F32, kind="ExternalOutput")
    kT_dram = nc.dram_tensor("kT_scratch", [HIDDEN, S], F16)

    with tile.TileContext(nc) as tc, ExitStack() as ctx:
        const = ctx.enter_context(tc.tile_pool(name="const", bufs=1))
        qpool = ctx.enter_context(tc.tile_pool(name="qrows", bufs=NIT))
        vpool = ctx.enter_context(tc.tile_pool(name="v16", bufs=NJT))
        avtp = ctx.enter_context(tc.tile_pool(name="avt", bufs=1))

        ident = const.tile([128, 128], F32)
        make_identity(nc, ident[:])

        # ---- replicated small constants ----
        def rep(name, src, n):
            t = const.tile([128, n], F32, tag=name)
            nc.sync.dma_start(t[:], src[0:1, :].broadcast_to([128, n]))
            return t

        bqRep = rep("bqRep", vecs["bq"], HIDDEN)
        bvRep = rep("bvRep", vecs["bv"], HIDDEN)
        gammaRep = rep("gammaRep", vecs["gamma"], HIDDEN)
        betaRep = rep("betaRep", vecs["beta"], HIDDEN)
        tempRep = rep("tempRep", temp, HEADS)
        negTemp = const.tile([128, HEADS], F32)
        nc.vector.tensor_scalar(out=negTemp[:], in0=tempRep[:], scalar1=-1.0,
                                scalar2=None, op0=OP.mult)
        bias72 = const.tile([128, HEADS], F32)
        nc.vector.tensor_scalar(out=bias72[:], in0=tempRep[:], scalar1=SHIFT,
                                scalar2=None, op0=OP.mult)

        # mask -> additive big-penalty row, replicated: (1-m)*1e6
        maskI = const.tile([128, S], I32, tag="maskstage")
        nc.sync.dma_start(maskI[:], maskin[0:1, :].broadcast_to([128, S]))
        maskBig = const.tile([128, S], F32)
        nc.vector.tensor_copy(maskBig[:], maskI[:])
        nc.vector.tensor_scalar(out=maskBig[:], in0=maskBig[:], scalar1=-30000.0,
                                scalar2=30000.0, op0=OP.mult, op1=OP.add)

        # per-partition bias columns for bk / bo: colT[p, c] = vec[c*128+p]
        def colT(name, src):
            t = const.tile([128, 4], F32, tag=name)
            nc.sync.dma_start(
                t[:], src.rearrange("a (c p) -> p (c a)", p=128))
            return t

        bkColT = colT("bkColT", vecs["bk"])
        boColT = colT("boColT", vecs["bo"])

        # ================= Phase A: projections =================
        WT = {}
        with tc.tile_pool(name="phA", bufs=2) as pa, \
             tc.tile_pool(name="phA_wnat", bufs=1) as pwn, \
             tc.tile_pool(name="phA_kT", bufs=1) as pkT, \
             tc.tile_pool(name="phA_ps", bufs=2, space="PSUM") as pps, \
             tc.tile_pool(name="phA_pst", bufs=2, space="PSUM") as ppt:

            # A0: load weights, build W^T (in-part, out-free), 4 chunks each
            for wn in ("Wq", "Wk", "Wv", "Wo"):
                wt_pool = const if wn == "Wo" else pwn
                wts = []
                for ic in range(4):
                    wt = wt_pool.tile([128, HIDDEN], F32, tag=f"{wn}T{ic}", name=f"{wn}T{ic}")
                    wts.append(wt)
                WT[wn] = wts
                for oc in range(4):
                    wnat = pa.tile([128, HIDDEN], F32, tag="wnat")
                    nc.sync.dma_start(wnat[:], Ws[wn][oc * 128:(oc + 1) * 128, :])
                    for ic in range(4):
                        pst = ppt.tile([128, 128], F32, tag="tpsA")
                        nc.tensor.transpose(
                            pst[:], wnat[:, ic * 128:(ic + 1) * 128], ident[:])
                        nc.scalar.copy(
                            WT[wn][ic][:, oc * 128:(oc + 1) * 128], pst[:])

            # A1: Q = query @ Wq.T + bq   -> Q tiles (i128, 512)
            Qt = []
            for it in range(NIT):
                qnat = pa.tile([128, HIDDEN], F32, tag="qnat")
                nc.sync.dma_start(qnat[:], query[it * 128:(it + 1) * 128, :])
                qTt = pa.tile([128, 128 * 4], F32, tag="qT")  # (in, i128) x 4chunks
                for ic in range(4):
                    pst = ppt.tile([128, 128], F32, tag="tpsA")
                    nc.tensor.transpose(
                        pst[:], qnat[:, ic * 128:(ic + 1) * 128], ident[:])
                    nc.scalar.copy(qTt[:, ic * 128:(ic + 1) * 128], pst[:])
                ps = pps.tile([128, HIDDEN], F32, tag="projps")
                for ic in range(4):
                    nc.tensor.matmul(ps[:], qTt[:, ic * 128:(ic + 1) * 128],
                                     WT["Wq"][ic][:],
                                     start=(ic == 0), stop=(ic == 3))
                q_sb = qpool.tile([128, HIDDEN], F32, tag="Q")
                nc.vector.tensor_tensor(out=q_sb[:], in0=ps[:], in1=bqRep[:],
                                        op=OP.add)
                Qt.append(q_sb)

            # A2: keyT chunks (in-part, j-free), then K^T = Wk@key^T + bk
            #     (+ mask fold) streamed to DRAM scratch
            keyT = [pkT.tile([128, S], F32, tag=f"keyT{ic}", name=f"keyT{ic}")
                    for ic in range(4)]
            for jt in range(NJT):
                knat = pa.tile([128, HIDDEN], F32, tag="knat")
                nc.sync.dma_start(knat[:], key[jt * 128:(jt + 1) * 128, :])
                for ic in range(4):
                    pst = ppt.tile([128, 128], F32, tag="tpsA")
                    nc.tensor.transpose(
                        pst[:], knat[:, ic * 128:(ic + 1) * 128], ident[:])
                    nc.scalar.copy(
                        keyT[ic][:, jt * 128:(jt + 1) * 128], pst[:])
            for oc in range(4):
                for jc in range(4):
                    ps = pps.tile([128, 512], F32, tag="projps")
                    for ic in range(4):
                        nc.tensor.matmul(
                            ps[:],
                            WT["Wk"][ic][:, oc * 128:(oc + 1) * 128],
                            keyT[ic][:, jc * 512:(jc + 1) * 512],
                            start=(ic == 0), stop=(ic == 3))
                    kT_sb = pa.tile([128, 512], F16, tag="kTsb")
                    nc.scalar.activation(kT_sb[:], ps[:], AF.Identity,
                                         bias=bkColT[:, oc:oc + 1])
                    nc.vector.tensor_tensor(
                        out=kT_sb[:], in0=kT_sb[:],
                        in1=maskBig[:, jc * 512:(jc + 1) * 512], op=OP.add)
                    nc.sync.dma_start(
                        kT_dram[oc * 128:(oc + 1) * 128,
                                jc * 512:(jc + 1) * 512], kT_sb[:])

            # A3: V (j-part, hd-free) fp16; reuse keyT slots for valueT
            valueT = [pkT.tile([128, S], F32, tag=f"keyT{ic}", name=f"valT{ic}")
                      for ic in range(4)]
            for jt in range(NJT):
                vnat = pa.tile([128, HIDDEN], F32, tag="knat")
                nc.sync.dma_start(vnat[:], value[jt * 128:(jt + 1) * 128, :])
                for ic in range(4):
                    pst = ppt.tile([128, 128], F32, tag="tpsA")
                    nc.tensor.transpose(
                        pst[:], vnat[:, ic * 128:(ic + 1) * 128], ident[:])
                    nc.scalar.copy(
                        valueT[ic][:, jt * 128:(jt + 1) * 128], pst[:])
            V16 = []
            for jt in range(NJT):
                ps = pps.tile([128, HIDDEN], F32, tag="projps")
                for ic in range(4):
                    nc.tensor.matmul(
                        ps[:],
                        valueT[ic][:, jt * 128:(jt + 1) * 128],
                        WT["Wv"][ic][:],
                        start=(ic == 0), stop=(ic == 3))
                v_sb = vpool.tile([128, HIDDEN], F16, tag="V16")
                nc.vector.tensor_tensor(out=v_sb[:], in0=ps[:], in1=bvRep[:],
                                        op=OP.add)
                V16.append(v_sb)

        # ============ Phases B+C per head: dist, softmax, attn, PV ============
        attnVT = [avtp.tile([128, SI], F32, tag=f"avt{c}", name=f"avt{c}")
              for c in range(4)]
        with tc.tile_pool(name="acc", bufs=1) as paccp, \
             tc.tile_pool(name="krep", bufs=4) as pkrep, \
             tc.tile_pool(name="tmp", bufs=3) as ptmp, \
             tc.tile_pool(name="sm", bufs=8) as psm, \
             tc.tile_pool(name="at16", bufs=6) as pat, \
             tc.tile_pool(name="psT", bufs=3, space="PSUM") as ppsT, \
             tc.tile_pool(name="psPV", bufs=2, space="PSUM") as ppsPV:
            for h in range(HEADS):
                acc = [paccp.tile([128, S], F32, tag=f"acc{it}", name=f"acc{it}")
                       for it in range(NIT)]
                acclo = [paccp.tile([128, S], F16, tag=f"acclo{it}",
                                    name=f"acclo{it}") for it in range(NIT)]
                # --- B: dist[i,j] = sum_d |Q[i,hd] - K[j,hd]| ---
                # fp16 partial sums over groups of 4 dims (DVE 2x mode),
                # flushed into the fp32 acc once per group.
                for d in range(HD):
                    hd = h * HD + d
                    g = d % 4
                    krep = pkrep.tile([128, S], F16, tag="krep")
                    nc.sync.dma_start(
                        krep[:], kT_dram[hd:hd + 1, :].broadcast_to([128, S]))
                    for it in range(NIT):
                        qcol = Qt[it][:, hd:hd + 1]
                        if g == 0:
                            nc.scalar.activation(acclo[it][:], krep[:], AF.Abs,
                                                 bias=qcol, scale=-1.0)
                        else:
                            tmp = ptmp.tile([128, S], F16, tag="tmp")
                            nc.scalar.activation(tmp[:], krep[:], AF.Abs,
                                                 bias=qcol, scale=-1.0)
                            nc.vector.tensor_tensor(
                                out=acclo[it][:], in0=acclo[it][:], in1=tmp[:],
                                op=OP.add)
                        if g == 3:
                            if d == 3:
                                nc.vector.tensor_copy(acc[it][:], acclo[it][:])
                            else:
                                nc.vector.tensor_tensor(
                                    out=acc[it][:], in0=acc[it][:],
                                    in1=acclo[it][:], op=OP.add)
                # --- C: softmax -> attn out; attn^T @ ... -> attnV^T ---
                for it in range(NIT):
                    denom = psm.tile([128, 1], F32, tag="denom")
                    nc.scalar.activation(acc[it][:], acc[it][:], AF.Exp,
                                         bias=bias72[:, h:h + 1],
                                         scale=negTemp[:, h:h + 1],
                                         accum_out=denom[:])
                    rden = psm.tile([128, 1], F32, tag="rden")
                    nc.vector.reciprocal(rden[:], denom[:])
                    nc.vector.tensor_scalar(out=acc[it][:], in0=acc[it][:],
                                            scalar1=rden[:, 0:1], scalar2=None,
                                            op0=OP.mult)
                    nc.sync.dma_start(
                        attn[h, it * 128:(it + 1) * 128, :], acc[it][:])
                    pv = ppsPV.tile([64, 128], F32, tag="pv")
                    for jt in range(NJT):
                        psT = ppsT.tile([128, 128], F32, tag="psT")
                        nc.tensor.transpose(
                            psT[:], acc[it][:, jt * 128:(jt + 1) * 128],
                            ident[:])
                        aT = pat.tile([128, 128], F16, tag="aT")
                        nc.scalar.copy(aT[:], psT[:])
                        nc.tensor.matmul(
                            pv[:], V16[jt][:, h * HD:(h + 1) * HD], aT[:],
                            start=(jt == 0), stop=(jt == NJT - 1))
                    c, r = (h * HD) // 128, (h * HD) % 128
                    nc.vector.tensor_copy(
                        attnVT[c][r:r + HD, it * 128:(it + 1) * 128], pv[:])

        # ============ Phase D: output proj + residual + layernorm ============
        with tc.tile_pool(name="phD", bufs=2) as pd, \
             tc.tile_pool(name="phD_oT", bufs=1) as poT, \
             tc.tile_pool(name="phD_sc", bufs=4) as psc, \
             tc.tile_pool(name="phD_ps", bufs=2, space="PSUM") as pdps, \
             tc.tile_pool(name="phD_pst", bufs=3, space="PSUM") as pdpt:
            # O^T tiles (o128, i512): O^T = Wo @ attnV^T (+bo)
            oT = []
            for oc in range(4):
                row = []
                for icn in range(2):
                    ps = pdps.tile([128, 512], F32, tag="ops")
                    for c in range(4):
                        nc.tensor.matmul(
                            ps[:],
                            WT["Wo"][c][:, oc * 128:(oc + 1) * 128],
                            attnVT[c][:, icn * 512:(icn + 1) * 512],
                            start=(c == 0), stop=(c == 3))
                    t = poT.tile([128, 512], F32, tag=f"oT{oc}_{icn}")
                    nc.scalar.activation(t[:], ps[:], AF.Identity,
                                         bias=boColT[:, oc:oc + 1])
                    row.append(t)
                oT.append(row)
            # transpose back to (i, hd), residual, LN
            for it in range(NIT):
                icn, sub = it // 4, it % 4
                res = pd.tile([128, HIDDEN], F32, tag="res")
                for oc in range(4):
                    pst = pdpt.tile([128, 128], F32, tag="tpsD")
                    nc.tensor.transpose(
                        pst[:], oT[oc][icn][:, sub * 128:(sub + 1) * 128],
                        ident[:])
                    nc.scalar.copy(res[:, oc * 128:(oc + 1) * 128], pst[:])
                qres = pd.tile([128, HIDDEN], F32, tag="qres")
                nc.sync.dma_start(qres[:], query[it * 128:(it + 1) * 128, :])
                nc.vector.tensor_tensor(out=res[:], in0=res[:], in1=qres[:],
                                        op=OP.add)
                # layernorm over hd
                scr = pd.tile([128, HIDDEN], F32, tag="scr")
                sx = psc.tile([128, 1], F32, tag="sx")
                nc.scalar.activation(scr[:], res[:], AF.Identity,
                                     accum_out=sx[:])
                sx2 = psc.tile([128, 1], F32, tag="sx2")
                nc.scalar.activation(scr[:], res[:], AF.Square,
                                     accum_out=sx2[:])
                mu = psc.tile([128, 1], F32, tag="mu")
                nc.vector.tensor_scalar(out=mu[:], in0=sx[:],
                                        scalar1=1.0 / HIDDEN, scalar2=None,
                                        op0=OP.mult)
                ex2 = psc.tile([128, 1], F32, tag="ex2")
                nc.vector.tensor_scalar(out=ex2[:], in0=sx2[:],
                                        scalar1=1.0 / HIDDEN, scalar2=None,
                                        op0=OP.mult)
                musq = psc.tile([128, 1], F32, tag="musq")
                nc.vector.tensor_scalar(out=musq[:], in0=mu[:],
                                        scalar1=mu[:, 0:1], scalar2=None,
                                        op0=OP.mult)
                var = psc.tile([128, 1], F32, tag="var")
                nc.vector.tensor_tensor(out=var[:], in0=ex2[:], in1=musq[:],
                                        op=OP.subtract)
                veps = psc.tile([128, 1], F32, tag="veps")
                nc.vector.tensor_scalar(out=veps[:], in0=var[:], scalar1=EPS,
                                        scalar2=None, op0=OP.add)
                std = psc.tile([128, 1], F32, tag="std")
                nc.scalar.activation(std[:], veps[:], AF.Sqrt)
                rstd = psc.tile([128, 1], F32, tag="rstd")
                nc.vector.reciprocal(rstd[:], std[:])
                nc.vector.tensor_scalar(out=res[:], in0=res[:],
                                        scalar1=mu[:, 0:1], scalar2=None,
                                        op0=OP.subtract)
                nc.vector.tensor_scalar(out=res[:], in0=res[:],
                                        scalar1=rstd[:, 0:1], scalar2=None,
                                        op0=OP.mult)
                nc.vector.tensor_tensor(out=res[:], in0=res[:], in1=gammaRep[:],
                                        op=OP.mult)
                nc.vector.tensor_tensor(out=res[:], in0=res[:], in1=betaRep[:],
                                        op=OP.add)
                nc.sync.dma_start(out[it * 128:(it + 1) * 128, :], res[:])

    nc.compile()
    return nc


_NC = None


def _get_nc():
    global _NC
    if _NC is None:
        _NC = build_kernel()
    return _NC


def make_in_maps(inputs):
    in_maps = []
    for c in range(8):
        b, half = c // 2, c % 2
        i0 = half * SI
        in_maps.append({
            "query": np.ascontiguousarray(inputs["query"][b, i0:i0 + SI]),
            "key": np.ascontiguousarray(inputs["key"][b]),
            "value": np.ascontiguousarray(inputs["value"][b]),
            "mask": np.ascontiguousarray(inputs["mask"][b][None, :]),
            "Wq": inputs["Wq"], "Wk": inputs["Wk"],
            "Wv": inputs["Wv"], "Wo": inputs["Wo"],
            "bq": inputs["bq"][None, :], "bk": inputs["bk"][None, :],
            "bv": inputs["bv"][None, :], "bo": inputs["bo"][None, :],
            "gamma": inputs["gamma"][None, :], "beta": inputs["beta"][None, :],
            "temperature": inputs["temperature"][None, :],
        })
    return in_maps


def kernel(**inputs):
    nc = _get_nc()
    res = run_bass_kernel_spmd(nc, make_in_maps(inputs), list(range(8)))
    out = np.empty((B, S, HIDDEN), np.float32)
    attn = np.empty((B, HEADS, S, S), np.float32)
    for c in range(8):
        b, half = c // 2, c % 2
        i0 = half * SI
        out[b, i0:i0 + SI] = res.results[c]["out"]
        attn[b, :, i0:i0 + SI, :] = res.results[c]["attn"]
    return out, attn


# revision 15
# speedup vs baseline: 1.1003x; 1.0037x over previous
"""MultiHeadManhattanAttention Trainium2 kernel (8-core SPMD).

Sharding: core c handles batch b = c//2 and query rows i in
[1024*(c%2), 1024*(c%2)+1024), all 8 heads. Everything (projections,
L1-distance scores, softmax, attn@V, output proj, layernorm) is
core-local; host only concatenates per-core outputs.

Layouts on chip:
  Q        (i128-part, hd-free)   per i-tile, bias columns for ACT
  K^T      DRAM scratch (512, 2048); per (h,d) row broadcast to 128
           partitions (krep) via stride-0 DMA
  dist/attn acc tiles (i128-part, j-free 2048), one per i-tile
  V        fp16 (j128-part, hd-free)
  attn^T   fp16 (j128-part, i-free) via PE transpose, feeds attn@V
  attnV^T  (hd-part, i-free) -> output proj -> PE transpose -> (i, hd)
"""
import sys
sys.path.insert(0, "/opt/trn_rl_repo")
import numpy as np
from contextlib import ExitStack

import concourse.bass as bass
from concourse import bacc
import concourse.tile as tile
import concourse.mybir as mybir
from concourse.bass_utils import run_bass_kernel_spmd
from concourse.masks import make_identity

HIDDEN, HEADS, HD = 512, 8, 64
B, S = 4, 2048
SI = 1024            # query rows per core
NIT = SI // 128      # 8 i-tiles
NJT = S // 128       # 16 j-tiles
EPS = 1e-5
SHIFT = 72.0         # softmax stability shift: exp(-temp*(dist-SHIFT))

F32 = mybir.dt.float32
F16 = mybir.dt.float16
I32 = mybir.dt.int32
AF = mybir.ActivationFunctionType
OP = mybir.AluOpType


def build_kernel():
    nc = bacc.Bacc("TRN2", target_bir_lowering=False, debug=False, num_devices=8)

    query = nc.dram_tensor("query", [SI, HIDDEN], F32, kind="ExternalInput")
    key = nc.dram_tensor("key", [S, HIDDEN], F32, kind="ExternalInput")
    value = nc.dram_tensor("value", [S, HIDDEN], F32, kind="ExternalInput")
    maskin = nc.dram_tensor("mask", [1, S], I32, kind="ExternalInput")
    Ws = {n: nc.dram_tensor(n, [HIDDEN, HIDDEN], F32, kind="ExternalInput")
          for n in ("Wq", "Wk", "Wv", "Wo")}
    vecs = {n: nc.dram_tensor(n, [1, HIDDEN], F32, kind="ExternalInput")
            for n in ("bq", "bk", "bv", "bo", "gamma", "beta")}
    temp = nc.dram_tensor("temperature", [1, HEADS], F32, kind="ExternalInput")

    out = nc.dram_tensor("out", [SI, HIDDEN], F32, kind="ExternalOutput")
    attn = nc.dram_tensor("attn", [HEADS, SI, S], F32, kind="ExternalOutput")
    kT_dram = nc.dram_tensor("kT_scratch", [HIDDEN, S], F16)

    with tile.TileContext(nc) as tc, ExitStack() as ctx:
        const = ctx.enter_context(tc.tile_pool(name="const", bufs=1))
        qpool = ctx.enter_context(tc.tile_pool(name="qrows", bufs=NIT))
        vpool = ctx.enter_context(tc.tile_pool(name="v16", bufs=NJT))
        avtp = ctx.enter_context(tc.tile_pool(name="avt", bufs=1))

        ident = const.tile([128, 128], F32)
        make_identity(nc, ident[:])

        # ---- replicated small constants ----
        def rep(name, src, n):
            t = const.tile([128, n], F32, tag=name)
            nc.sync.dma_start(t[:], src[0:1, :].broadcast_to([128, n]))
            return t

        bqRep = rep("bqRep", vecs["bq"], HIDDEN)
        bvRep = rep("bvRep", vecs["bv"], HIDDEN)
        gammaRep = rep("gammaRep", vecs["gamma"], HIDDEN)
        betaRep = rep("betaRep", vecs["beta"], HIDDEN)
        tempRep = rep("tempRep", temp, HEADS)
        negTemp = const.tile([128, HEADS], F32)
        nc.vector.tensor_scalar(out=negTemp[:], in0=tempRep[:], scalar1=-1.0,
                                scalar2=None, op0=OP.mult)
        bias72 = const.tile([128, HEADS], F32)
        nc.vector.tensor_scalar(out=bias72[:], in0=tempRep[:], scalar1=SHIFT,
                                scalar2=None, op0=OP.mult)

        # mask -> additive big-penalty row, replicated: (1-m)*1e6
        maskI = const.tile([128, S], I32, tag="maskstage")
        nc.sync.dma_start(maskI[:], maskin[0:1, :].broadcast_to([128, S]))
        maskBig = const.tile([128, S], F32)
        nc.vector.tensor_copy(maskBig[:], maskI[:])
        nc.vector.tensor_scalar(out=maskBig[:], in0=maskBig[:], scalar1=-30000.0,
                                scalar2=30000.0, op0=OP.mult, op1=OP.add)

        # per-partition bias columns for bk / bo: colT[p, c] = vec[c*128+p]
        def colT(name, src):
            t = const.tile([128, 4], F32, tag=name)
            nc.sync.dma_start(
                t[:], src.rearrange("a (c p) -> p (c a)", p=128))
            return t

        bkColT = colT("bkColT", vecs["bk"])
        boColT = colT("boColT", vecs["bo"])

        # ================= Phase A: projections =================
        WT = {}
        with tc.tile_pool(name="phA", bufs=2) as pa, \
             tc.tile_pool(name="phA_wnat", bufs=1) as pwn, \
             tc.tile_pool(name="phA_kT", bufs=1) as pkT, \
             tc.tile_pool(name="phA_ps", bufs=2, space="PSUM") as pps, \
             tc.tile_pool(name="phA_pst", bufs=2, space="PSUM") as ppt:

            # A0: load weights, build W^T (in-part, out-free), 4 chunks each
            for wn in ("Wq", "Wk", "Wv", "Wo"):
                wt_pool = const if wn == "Wo" else pwn
                wts = []
                for ic in range(4):
                    wt = wt_pool.tile([128, HIDDEN], F32, tag=f"{wn}T{ic}", name=f"{wn}T{ic}")
                    wts.append(wt)
                WT[wn] = wts
                for oc in range(4):
                    wnat = pa.tile([128, HIDDEN], F32, tag="wnat")
                    nc.sync.dma_start(wnat[:], Ws[wn][oc * 128:(oc + 1) * 128, :])
                    for ic in range(4):
                        pst = ppt.tile([128, 128], F32, tag="tpsA")
                        nc.tensor.transpose(
                            pst[:], wnat[:, ic * 128:(ic + 1) * 128], ident[:])
                        nc.vector.tensor_copy(
                            WT[wn][ic][:, oc * 128:(oc + 1) * 128], pst[:])

            # A1: Q = query @ Wq.T + bq   -> Q tiles (i128, 512)
            Qt = []
            for it in range(NIT):
                qnat = pa.tile([128, HIDDEN], F32, tag="qnat")
                nc.sync.dma_start(qnat[:], query[it * 128:(it + 1) * 128, :])
                qTt = pa.tile([128, 128 * 4], F32, tag="qT")  # (in, i128) x 4chunks
                for ic in range(4):
                    pst = ppt.tile([128, 128], F32, tag="tpsA")
                    nc.tensor.transpose(
                        pst[:], qnat[:, ic * 128:(ic + 1) * 128], ident[:])
                    nc.vector.tensor_copy(qTt[:, ic * 128:(ic + 1) * 128], pst[:])
                ps = pps.tile([128, HIDDEN], F32, tag="projps")
                for ic in range(4):
                    nc.tensor.matmul(ps[:], qTt[:, ic * 128:(ic + 1) * 128],
                                     WT["Wq"][ic][:],
                                     start=(ic == 0), stop=(ic == 3))
                q_sb = qpool.tile([128, HIDDEN], F32, tag="Q")
                nc.vector.tensor_tensor(out=q_sb[:], in0=ps[:], in1=bqRep[:],
                                        op=OP.add)
                Qt.append(q_sb)

            # A2: keyT chunks (in-part, j-free), then K^T = Wk@key^T + bk
            #     (+ mask fold) streamed to DRAM scratch
            keyT = [pkT.tile([128, S], F32, tag=f"keyT{ic}", name=f"keyT{ic}")
                    for ic in range(4)]
            for jt in range(NJT):
                knat = pa.tile([128, HIDDEN], F32, tag="knat")
                nc.sync.dma_start(knat[:], key[jt * 128:(jt + 1) * 128, :])
                for ic in range(4):
                    pst = ppt.tile([128, 128], F32, tag="tpsA")
                    nc.tensor.transpose(
                        pst[:], knat[:, ic * 128:(ic + 1) * 128], ident[:])
                    nc.vector.tensor_copy(
                        keyT[ic][:, jt * 128:(jt + 1) * 128], pst[:])
            for oc in range(4):
                for jc in range(4):
                    ps = pps.tile([128, 512], F32, tag="projps")
                    for ic in range(4):
                        nc.tensor.matmul(
                            ps[:],
                            WT["Wk"][ic][:, oc * 128:(oc + 1) * 128],
                            keyT[ic][:, jc * 512:(jc + 1) * 512],
                            start=(ic == 0), stop=(ic == 3))
                    kT_sb = pa.tile([128, 512], F16, tag="kTsb")
                    nc.vector.tensor_scalar(out=kT_sb[:], in0=ps[:],
                                            scalar1=bkColT[:, oc:oc + 1],
                                            scalar2=None, op0=OP.add)
                    nc.vector.tensor_tensor(
                        out=kT_sb[:], in0=kT_sb[:],
                        in1=maskBig[:, jc * 512:(jc + 1) * 512], op=OP.add)
                    nc.sync.dma_start(
                        kT_dram[oc * 128:(oc + 1) * 128,
                                jc * 512:(jc + 1) * 512], kT_sb[:])

            # A3: V (j-part, hd-free) fp16; reuse keyT slots for valueT
            valueT = [pkT.tile([128, S], F32, tag=f"keyT{ic}", name=f"valT{ic}")
                      for ic in range(4)]
            for jt in range(NJT):
                vnat = pa.tile([128, HIDDEN], F32, tag="knat")
                nc.sync.dma_start(vnat[:], value[jt * 128:(jt + 1) * 128, :])
                for ic in range(4):
                    pst = ppt.tile([128, 128], F32, tag="tpsA")
                    nc.tensor.transpose(
                        pst[:], vnat[:, ic * 128:(ic + 1) * 128], ident[:])
                    nc.vector.tensor_copy(
                        valueT[ic][:, jt * 128:(jt + 1) * 128], pst[:])
            V16 = []
            for jt in range(NJT):
                ps = pps.tile([128, HIDDEN], F32, tag="projps")
                for ic in range(4):
                    nc.tensor.matmul(
                        ps[:],
                        valueT[ic][:, jt * 128:(jt + 1) * 128],
                        WT["Wv"][ic][:],
                        start=(ic == 0), stop=(ic == 3))
                v_sb = vpool.tile([128, HIDDEN], F16, tag="V16")
                nc.vector.tensor_tensor(out=v_sb[:], in0=ps[:], in1=bvRep[:],
                                        op=OP.add)
                V16.append(v_sb)

        # ============ Phases B+C per head: dist, softmax, attn, PV ============
        attnVT = [avtp.tile([128, SI], F32, tag=f"avt{c}", name=f"avt{c}")
              for c in range(4)]
        with tc.tile_pool(name="acc", bufs=1) as paccp, \
             tc.tile_pool(name="krep", bufs=4) as pkrep, \
             tc.tile_pool(name="tmp", bufs=3) as ptmp, \
             tc.tile_pool(name="sm", bufs=8) as psm, \
             tc.tile_pool(name="at16", bufs=8) as pat, \
             tc.tile_pool(name="psT", bufs=4, space="PSUM") as ppsT, \
             tc.tile_pool(name="psPV", bufs=2, space="PSUM") as ppsPV:
            for h in range(HEADS):
                acc = [paccp.tile([128, S], F32, tag=f"acc{it}", name=f"acc{it}")
                       for it in range(NIT)]
                acclo = [paccp.tile([128, S], F16, tag=f"acclo{it}",
                                    name=f"acclo{it}") for it in range(NIT)]
                # --- B: dist[i,j] = sum_d |Q[i,hd] - K[j,hd]| ---
                # fp16 partial sums over groups of 4 dims (DVE 2x mode),
                # flushed into the fp32 acc once per group.
                for d in range(HD):
                    hd = h * HD + d
                    g = d % 4
                    krep = pkrep.tile([128, S], F16, tag="krep")
                    nc.sync.dma_start(
                        krep[:], kT_dram[hd:hd + 1, :].broadcast_to([128, S]))
                    for it in range(NIT):
                        qcol = Qt[it][:, hd:hd + 1]
                        if g == 0:
                            nc.scalar.activation(acclo[it][:], krep[:], AF.Abs,
                                                 bias=qcol, scale=-1.0)
                        else:
                            tmp = ptmp.tile([128, S], F16, tag="tmp")
                            nc.scalar.activation(tmp[:], krep[:], AF.Abs,
                                                 bias=qcol, scale=-1.0)
                            nc.vector.tensor_tensor(
                                out=acclo[it][:], in0=acclo[it][:], in1=tmp[:],
                                op=OP.add)
                        if g == 3:
                            if d == 3:
                                nc.vector.tensor_copy(acc[it][:], acclo[it][:])
                            else:
                                nc.vector.tensor_tensor(
                                    out=acc[it][:], in0=acc[it][:],
                                    in1=acclo[it][:], op=OP.add)
                # --- C: softmax -> attn out; attn^T @ ... -> attnV^T ---
                for it in range(NIT):
                    denom = psm.tile([128, 1], F32, tag="denom")
                    nc.scalar.activation(acc[it][:], acc[it][:], AF.Exp,
                                         bias=bias72[:, h:h + 1],
                                         scale=negTemp[:, h:h + 1],
                                         accum_out=denom[:])
                    rden = psm.tile([128, 1], F32, tag="rden")
                    nc.vector.reciprocal(rden[:], denom[:])
                    nc.vector.tensor_scalar(out=acc[it][:], in0=acc[it][:],
                                            scalar1=rden[:, 0:1], scalar2=None,
                                            op0=OP.mult)
                    nc.sync.dma_start(
                        attn[h, it * 128:(it + 1) * 128, :], acc[it][:])
                    pv = ppsPV.tile([64, 128], F32, tag="pv")
                    for jt in range(NJT):
                        psT = ppsT.tile([128, 128], F32, tag="psT")
                        nc.tensor.transpose(
                            psT[:], acc[it][:, jt * 128:(jt + 1) * 128],
                            ident[:])
                        aT = pat.tile([128, 128], F16, tag="aT")
                        nc.vector.tensor_copy(aT[:], psT[:])
                        nc.tensor.matmul(
                            pv[:], V16[jt][:, h * HD:(h + 1) * HD], aT[:],
                            start=(jt == 0), stop=(jt == NJT - 1))
                    c, r = (h * HD) // 128, (h * HD) % 128
                    nc.vector.tensor_copy(
                        attnVT[c][r:r + HD, it * 128:(it + 1) * 128], pv[:])

        # ============ Phase D: output proj + residual + layernorm ============
        with tc.tile_pool(name="phD", bufs=2) as pd, \
             tc.tile_pool(name="phD_oT", bufs=1) as poT, \
             tc.tile_pool(name="phD_sc", bufs=4) as psc, \
             tc.tile_pool(name="phD_ps", bufs=2, space="PSUM") as pdps, \
             tc.tile_pool(name="phD_pst", bufs=3, space="PSUM") as pdpt:
            # O^T tiles (o128, i512): O^T = Wo @ attnV^T (+bo)
            oT = []
            for oc in range(4):
                row = []
                for icn in range(2):
                    ps = pdps.tile([128, 512], F32, tag="ops")
                    for c in range(4):
                        nc.tensor.matmul(
                            ps[:],
                            WT["Wo"][c][:, oc * 128:(oc + 1) * 128],
                            attnVT[c][:, icn * 512:(icn + 1) * 512],
                            start=(c == 0), stop=(c == 3))
                    t = poT.tile([128, 512], F32, tag=f"oT{oc}_{icn}")
                    nc.vector.tensor_scalar(out=t[:], in0=ps[:],
                                            scalar1=boColT[:, oc:oc + 1],
                                            scalar2=None, op0=OP.add)
                    row.append(t)
                oT.append(row)
            # transpose back to (i, hd), residual, LN
            for it in range(NIT):
                icn, sub = it // 4, it % 4
                res = pd.tile([128, HIDDEN], F32, tag="res")
                for oc in range(4):
                    pst = pdpt.tile([128, 128], F32, tag="tpsD")
                    nc.tensor.transpose(
                        pst[:], oT[oc][icn][:, sub * 128:(sub + 1) * 128],
                        ident[:])
                    nc.vector.tensor_copy(res[:, oc * 128:(oc + 1) * 128], pst[:])
                qres = pd.tile([128, HIDDEN], F32, tag="qres")
                nc.sync.dma_start(qres[:], query[it * 128:(it + 1) * 128, :])
                nc.vector.tensor_tensor(out=res[:], in0=res[:], in1=qres[:],
                                        op=OP.add)
                # layernorm over hd
                scr = pd.tile([128, HIDDEN], F32, tag="scr")
                sx = psc.tile([128, 1], F32, tag="sx")
                nc.scalar.activation(scr[:], res[:], AF.Identity,
                                     accum_out=sx[:])
                sx2 = psc.tile([128, 1], F32, tag="sx2")
                nc.scalar.activation(scr[:], res[:], AF.Square,
                                     accum_out=sx2[:])
                mu = psc.tile([128, 1], F32, tag="mu")
                nc.vector.tensor_scalar(out=mu[:], in0=sx[:],
                                        scalar1=1.0 / HIDDEN, scalar2=None,
                                        op0=OP.mult)
                ex2 = psc.tile([128, 1], F32, tag="ex2")
                nc.vector.tensor_scalar(out=ex2[:], in0=sx2[:],
                                        scalar1=1.0 / HIDDEN, scalar2=None,
                                        op0=OP.mult)
                musq = psc.tile([128, 1], F32, tag="musq")
                nc.vector.tensor_scalar(out=musq[:], in0=mu[:],
                                        scalar1=mu[:, 0:1], scalar2=None,
                                        op0=OP.mult)
                var = psc.tile([128, 1], F32, tag="var")
                nc.vector.tensor_tensor(out=var[:], in0=ex2[:], in1=musq[:],
                                        op=OP.subtract)
                veps = psc.tile([128, 1], F32, tag="veps")
                nc.vector.tensor_scalar(out=veps[:], in0=var[:], scalar1=EPS,
                                        scalar2=None, op0=OP.add)
                std = psc.tile([128, 1], F32, tag="std")
                nc.scalar.activation(std[:], veps[:], AF.Sqrt)
                rstd = psc.tile([128, 1], F32, tag="rstd")
                nc.vector.reciprocal(rstd[:], std[:])
                nc.vector.tensor_scalar(out=res[:], in0=res[:],
                                        scalar1=mu[:, 0:1], scalar2=None,
                                        op0=OP.subtract)
                nc.vector.tensor_scalar(out=res[:], in0=res[:],
                                        scalar1=rstd[:, 0:1], scalar2=None,
                                        op0=OP.mult)
                nc.vector.tensor_tensor(out=res[:], in0=res[:], in1=gammaRep[:],
                                        op=OP.mult)
                nc.vector.tensor_tensor(out=res[:], in0=res[:], in1=betaRep[:],
                                        op=OP.add)
                nc.sync.dma_start(out[it * 128:(it + 1) * 128, :], res[:])

    nc.compile()
    return nc


_NC = None


def _get_nc():
    global _NC
    if _NC is None:
        _NC = build_kernel()
    return _NC


def make_in_maps(inputs):
    in_maps = []
    for c in range(8):
        b, half = c // 2, c % 2
        i0 = half * SI
        in_maps.append({
            "query": np.ascontiguousarray(inputs["query"][b, i0:i0 + SI]),
            "key": np.ascontiguousarray(inputs["key"][b]),
            "value": np.ascontiguousarray(inputs["value"][b]),
            "mask": np.ascontiguousarray(inputs["mask"][b][None, :]),
            "Wq": inputs["Wq"], "Wk": inputs["Wk"],
            "Wv": inputs["Wv"], "Wo": inputs["Wo"],
            "bq": inputs["bq"][None, :], "bk": inputs["bk"][None, :],
            "bv": inputs["bv"][None, :], "bo": inputs["bo"][None, :],
            "gamma": inputs["gamma"][None, :], "beta": inputs["beta"][None, :],
            "temperature": inputs["temperature"][None, :],
        })
    return in_maps


def kernel(**inputs):
    nc = _get_nc()
    res = run_bass_kernel_spmd(nc, make_in_maps(inputs), list(range(8)))
    out = np.empty((B, S, HIDDEN), np.float32)
    attn = np.empty((B, HEADS, S, S), np.float32)
    for c in range(8):
        b, half = c // 2, c % 2
        i0 = half * SI
        out[b, i0:i0 + SI] = res.results[c]["out"]
        attn[b, :, i0:i0 + SI, :] = res.results[c]["attn"]
    return out, attn
